# revision 18
# baseline (speedup 1.0000x reference)
"""GraphSAGE v4: dma_gather-based gathers + bf16 allgather.

Cross-core dedup of layer-1 rows (as v2): the 45,056 phase-2 refs are
deduped host-side to a sorted-unique list padded to 8 x SH rows; core c
computes h1 for its SH-row block.

Gathers use InstDMAGatherAnt (int16 indices, 16-partition-wrapped): the
SWDGE fixed cost (~1us) amortizes over thousands of descriptors, unlike
indirect_dma_start which on HW honors only one index per partition.
Phase-1 indices don't fit int16 against the 100K-row table, so the host
packs each 4-tile group's unique rows into a fixed 5632-row segment of a
per-core "bigtable" and rewrites indices group-locally (< 5632).

h1 is stored bf16: halves allgather bytes (CC fixed ~5us/chunk,
~158 GB/s) and phase-2 gather traffic. Aggregations run as one strided
tensor_reduce per tile on DVE; PSUM->SBUF copies go to the scalar
engine. Final rel err ~1e-3 (bf16 h1), well under the 2e-2 gate.
"""

import sys

for _p in ("/opt/trn_rl_repo", "/root/.axon_site/_ro/trn_rl_repo"):
    if _p not in sys.path:
        sys.path.insert(0, _p)

import numpy as np

import concourse.bass as bass
import concourse.mybir as mybir
import concourse.tile as tile
from concourse import bacc
from concourse.bass_utils import run_bass_kernel_spmd

N, D, OUT, K = 100000, 256, 128, 10
N1, B = 40960, 4096
NCORES = 8
BC = B // NCORES                 # 512 batch rows per core
NREF = BC * (K + 1)              # 5632 phase-2 refs
TR = NREF // 128                 # 44 phase-2 gather columns
T2 = BC // 128                   # 4 output tiles
K1 = K + 1
G = 4                            # phase-1 tiles per gather group
SEG = G * 128 * K1               # 5632: padded rows per bigtable segment
W1G = SEG // 16                  # 352: idx cols per full group (16-wrap)

_CACHE = {}


def _chunk_schedule(sh):
    """Allgather chunks (rows): ~1024-row chunks with a 512-row tail so
    the last chunk's serial latency (phase 2 waits on it) stays small."""
    chunks = []
    rem = sh
    while rem > 1536:
        chunks.append(1024)
        rem -= 1024
    if rem > 512:
        chunks.append(rem - 512)
        rem = 512
    chunks.append(rem)
    assert sum(chunks) == sh
    return tuple(chunks)


def _build(SH):
    T1 = SH // 128
    U = SH * NCORES
    NG = -(-T1 // G)
    CHUNKS = _chunk_schedule(SH)
    CH_START = tuple(sum(CHUNKS[:i]) for i in range(len(CHUNKS)))
    f32 = mybir.dt.float32
    bf16 = mybir.dt.bfloat16
    i16 = mybir.dt.int16
    nc = bacc.Bacc("TRN2", target_bir_lowering=False, debug=False,
                   num_devices=NCORES)
    bigtable = nc.dram_tensor("bigtable", [NG * SEG, D], f32,
                              kind="ExternalInput").ap()
    idx1 = nc.dram_tensor("idx1", [128, NG * W1G], i16,
                          kind="ExternalInput").ap()
    idx2 = nc.dram_tensor("idx2", [128, NREF // 16], i16,
                          kind="ExternalInput").ap()
    w1p = nc.dram_tensor("w1p", [2 * D, OUT], f32, kind="ExternalInput").ap()
    w2p = nc.dram_tensor("w2p", [2 * OUT, OUT], f32, kind="ExternalInput").ap()
    ident = nc.dram_tensor("ident", [128, 128], f32, kind="ExternalInput").ap()
    out = nc.dram_tensor("out", [BC, OUT], f32, kind="ExternalOutput").ap()
    shard = nc.dram_tensor("shard", [SH, OUT], bf16)
    h1all = nc.dram_tensor("h1all", [U, OUT], bf16, addr_space="Shared")

    relu = mybir.ActivationFunctionType.Relu

    with tile.TileContext(nc) as tc:
        with tc.tile_pool(name="const", bufs=1) as constp, \
             tc.tile_pool(name="gat", bufs=2) as gatp, \
             tc.tile_pool(name="agg", bufs=4) as aggp, \
             tc.tile_pool(name="xt", bufs=8) as xtp, \
             tc.tile_pool(name="g2", bufs=1) as g2p, \
             tc.tile_pool(name="ps", bufs=3, space="PSUM") as psp, \
             tc.tile_pool(name="psb", bufs=1, space="PSUM") as psbp, \
             tc.tile_pool(name="psh", bufs=2, space="PSUM") as pshp, \
             tc.tile_pool(name="o", bufs=4) as outp:

            # index tiles load first: HWDGE runs in program order per
            # engine, and the first gather only needs idx1's first group
            idx1_t = constp.tile([128, NG * W1G], i16, tag="idx1")
            nc.sync.dma_start(out=idx1_t[:, :W1G], in_=idx1[:, :W1G])
            if NG > 1:
                nc.sync.dma_start(out=idx1_t[:, W1G:], in_=idx1[:, W1G:])
            idn = constp.tile([128, 128], f32)
            nc.sync.dma_start(out=idn[:], in_=ident[:])
            idnb = constp.tile([128, 128], bf16, tag="idnb")
            nc.vector.tensor_copy(out=idnb[:], in_=idn[:])
            w1t = constp.tile([128, 4 * OUT], f32, tag="w1")
            for c in range(4):
                nc.sync.dma_start(out=w1t[:, c * OUT:(c + 1) * OUT],
                                  in_=w1p[c * 128:(c + 1) * 128, :])
            w2t = constp.tile([128, 2 * OUT], f32, tag="w2")
            for c in range(2):
                nc.sync.dma_start(out=w2t[:, c * OUT:(c + 1) * OUT],
                                  in_=w2p[c * 128:(c + 1) * 128, :])
            w2tb = constp.tile([128, 2 * OUT], bf16, tag="w2b")
            nc.vector.tensor_copy(out=w2tb[:], in_=w2t[:])
            idx2_t = constp.tile([128, NREF // 16], i16, tag="idx2")
            nc.sync.dma_start(out=idx2_t[:], in_=idx2[:, :])

            # ---- phase 1: compute node-major h1 shard -> DRAM ----
            for g in range(NG):
                gn = min(G, T1 - g * G)
                ncols = gn * K1
                g3 = gatp.tile([128, G * K1, D], f32)
                # HW caps one gather at 1024 indices (SWDGE desc scratch
                # split across the 8 Q7 cores) -> chunks of 8 columns
                for c0 in range(0, ncols, 8):
                    c1 = min(ncols, c0 + 8)
                    nc.gpsimd.dma_gather(
                        out_ap=g3[:, c0:c1, :],
                        in_ap=bigtable[g * SEG:(g + 1) * SEG, :],
                        idxs_ap=idx1_t[:, g * W1G + c0 * 8:g * W1G + c1 * 8],
                        num_idxs=(c1 - c0) * 128,
                        num_idxs_reg=(c1 - c0) * 128,
                        elem_size=D,
                    )
                for tl in range(gn):
                    t = g * G + tl
                    self2d = g3[:, tl * K1, :]
                    nbv = g3[:, tl * K1 + 1:(tl + 1) * K1, :].transpose(
                        [0, 2, 1])
                    a = aggp.tile([128, D], f32)
                    nc.vector.tensor_reduce(a[:], nbv, mybir.AxisListType.X,
                                            mybir.AluOpType.add)
                    srcs = (self2d[:, 0:128], self2d[:, 128:256],
                            a[:, 0:128], a[:, 128:256])
                    psum_h = pshp.tile([128, 128], f32, space="PSUM")
                    for c, src in enumerate(srcs):
                        pt = psp.tile([128, 128], f32, space="PSUM", tag="tp")
                        nc.tensor.transpose(out=pt[:], in_=src, identity=idn[:])
                        xt = xtp.tile([128, 128], f32, tag=f"xt{c}")
                        nc.scalar.copy(out=xt[:], in_=pt[:])
                        # node-major: out[nodes, outf] = xt.T @ w1chunk
                        nc.tensor.matmul(out=psum_h[:],
                                         lhsT=xt[:],
                                         rhs=w1t[:, c * OUT:(c + 1) * OUT],
                                         start=(c == 0), stop=(c == 3))
                    ho = outp.tile([128, OUT], bf16, tag="ho")
                    nc.scalar.activation(ho[:], psum_h[:], relu)
                    nc.sync.dma_start(out=shard[t * 128:(t + 1) * 128, :],
                                      in_=ho[:])
                    # chunk finished? -> allgather it (overlaps later tiles)
                    done = (t + 1) * 128
                    for (s, L) in zip(CH_START, CHUNKS):
                        if s + L == done:
                            nc.gpsimd.collective_compute(
                                "AllGather", mybir.AluOpType.bypass,
                                replica_groups=[list(range(NCORES))],
                                ins=[shard[s:s + L, :]],
                                outs=[h1all[s * NCORES:(s + L) * NCORES, :]],
                            )

            # ---- phase 2: gather h1 rows, aggregate, second layer ----
            # columns: [self(T2) | neighbors k-major (K*T2)]
            g2all = g2p.tile([128, TR, OUT], bf16, tag="g2all")
            for c0 in range(0, TR, 8):
                c1 = min(TR, c0 + 8)
                nc.gpsimd.dma_gather(
                    out_ap=g2all[:, c0:c1, :],
                    in_ap=h1all[:, :],
                    idxs_ap=idx2_t[:, c0 * 8:c1 * 8],
                    num_idxs=(c1 - c0) * 128,
                    num_idxs_reg=(c1 - c0) * 128,
                    elem_size=OUT,
                )
            g2v = g2all[:, :, :].rearrange("p (k2 t) o -> p k2 t o", k2=K1)
            for t in range(T2):
                a2 = aggp.tile([128, OUT], f32, tag="a2")
                nbv2 = g2v[:, 1:, t, :].transpose([0, 2, 1])
                nc.vector.tensor_reduce(a2[:], nbv2, mybir.AxisListType.X,
                                        mybir.AluOpType.add)
                self2 = g2v[:, 0, t, :]
                ps2 = pshp.tile([128, 128], f32, space="PSUM", tag="ps2")
                st = psbp.tile([128, 128], bf16, space="PSUM", tag="tpb")
                nc.tensor.transpose(out=st[:], in_=self2, identity=idnb[:])
                s2t = xtp.tile([128, 128], bf16, tag="s2t")
                nc.scalar.copy(out=s2t[:], in_=st[:])
                at = psp.tile([128, 128], f32, space="PSUM", tag="tp")
                nc.tensor.transpose(out=at[:], in_=a2[:], identity=idn[:])
                a2t = xtp.tile([128, 128], bf16, tag="a2t")
                nc.scalar.copy(out=a2t[:], in_=at[:])
                nc.tensor.matmul(out=ps2[:], lhsT=s2t[:], rhs=w2tb[:, 0:OUT],
                                 start=True, stop=False)
                nc.tensor.matmul(out=ps2[:], lhsT=a2t[:],
                                 rhs=w2tb[:, OUT:2 * OUT],
                                 start=False, stop=True)
                o = outp.tile([128, OUT], f32, tag="o2")
                nc.scalar.activation(o[:], ps2[:], relu)
                nc.sync.dma_start(out=out[t * 128:(t + 1) * 128, :], in_=o[:])

    nc.compile()
    return nc


def _wrap16(idxlist):
    """int16 index list -> [128, len/16] array with the 16-partition wrap
    the gather ucode expects: idx[p, s] = idxlist[s*16 + p], and the
    [16, W] pattern REPLICATED across all 8 Q7-core stripes (HW reads
    each core's own 16-partition stripe; zeros there silently gather
    row 0 instead)."""
    n = len(idxlist)
    assert n % 16 == 0
    w16 = idxlist.astype(np.int16).reshape(n // 16, 16).T
    return np.ascontiguousarray(np.tile(w16, (8, 1)))


def _prep_inputs(raw_features, W1, W2, nodes1, neighs1, map2, neighs2):
    raw = np.ascontiguousarray(np.asarray(raw_features, dtype=np.float32))
    W1 = np.asarray(W1, dtype=np.float32)
    W2 = np.asarray(W2, dtype=np.float32)
    nodes1 = np.asarray(nodes1).astype(np.int64)
    neighs1 = np.asarray(neighs1).astype(np.int64)
    map2 = np.asarray(map2).astype(np.int64)
    neighs2 = np.asarray(neighs2).astype(np.int64)

    w1p = np.concatenate([W1[:, :D], W1[:, D:] * (1.0 / K)], axis=1).T
    w2p = np.concatenate([W2[:, :OUT], W2[:, OUT:] * (1.0 / K)], axis=1).T
    w1p = np.ascontiguousarray(w1p, dtype=np.float32)
    w2p = np.ascontiguousarray(w2p, dtype=np.float32)
    ident = np.eye(128, dtype=np.float32)

    # global dedup of layer-1 rows over ALL cores; size the shard to fit
    refs = np.concatenate([map2, neighs2.reshape(-1)])      # [45056]
    uniq, inv = np.unique(refs, return_inverse=True)
    ua = len(uniq)
    SH = -(-ua // (NCORES * 128)) * 128  # per-core rows, 128-tile padded
    T1 = SH // 128
    U = SH * NCORES
    NG = -(-T1 // G)
    CHUNKS = _chunk_schedule(SH)
    CH_START = tuple(sum(CHUNKS[:i]) for i in range(len(CHUNKS)))
    uniq_pad = np.concatenate([uniq, np.zeros(U - ua, dtype=uniq.dtype)])
    # position of unique index u in the chunk-interleaved allgather layout
    cidx = np.arange(U) // SH            # owning core
    r = np.arange(U) % SH                # row within core shard
    starts = np.asarray(CH_START)
    sizes = np.asarray(CHUNKS)
    j = np.searchsorted(starts, r, side="right") - 1        # chunk id
    pos_of_u = starts[j] * NCORES + cidx * sizes[j] + (r - starts[j])

    in_maps = []
    for c in range(NCORES):
        # phase-1 raw row ids for this core's unique block: [SH, 11]
        blk = uniq_pad[c * SH:(c + 1) * SH]
        idsmat = np.stack([nodes1[blk]] + [neighs1[blk, k] for k in range(K)],
                          axis=1)                            # [SH, K1] int64
        # per-group compaction: unique rows -> fixed 5632-row segment,
        # indices rewritten group-local so they fit int16
        bigtable = np.zeros((NG * SEG, D), dtype=np.float32)
        idx1 = np.zeros((128, NG * W1G), dtype=np.int16)
        for g in range(NG):
            t0, t1 = g * G, min(T1, (g + 1) * G)
            gn = t1 - t0
            grefs = idsmat[t0 * 128:t1 * 128]                # [gn*128, K1]
            guniq, ginv = np.unique(grefs, return_inverse=True)
            assert len(guniq) <= SEG
            bigtable[g * SEG:g * SEG + len(guniq)] = raw[guniq]
            # gather order: (tile-local, k, partition)
            il = ginv.reshape(gn, 128, K1).transpose(0, 2, 1).reshape(-1)
            idx1[:, g * W1G:g * W1G + gn * K1 * 8] = _wrap16(il)
        # phase-2 refs: self (BC) then neighbors k-major (K x BC)
        sl = slice(c * BC, (c + 1) * BC)
        self_u = inv[np.arange(B)[sl]]                      # into uniq
        neigh_u = inv[B + (np.arange(c * BC * K, (c + 1) * BC * K)
                           .reshape(BC, K))]                # [BC, K]
        l2 = np.concatenate([pos_of_u[self_u],
                             pos_of_u[neigh_u.T.reshape(-1)]])
        idx2 = _wrap16(l2)
        in_maps.append({"bigtable": bigtable, "idx1": idx1, "idx2": idx2,
                        "w1p": w1p, "w2p": w2p, "ident": ident})
    return SH, in_maps


def run(inputs: dict, trace: bool = False):
    SH, in_maps = _prep_inputs(**inputs)
    if SH not in _CACHE:
        _CACHE[SH] = _build(SH)
    nc = _CACHE[SH]
    try:
        res = run_bass_kernel_spmd(nc, in_maps,
                                   core_ids=list(range(NCORES)), trace=trace)
    except Exception:
        # transient device wedge (e.g. NRT_EXEC_UNIT_UNRECOVERABLE) --
        # a single retry has been sufficient in practice
        res = run_bass_kernel_spmd(nc, in_maps,
                                   core_ids=list(range(NCORES)), trace=trace)
    outp = np.concatenate([res.results[c]["out"] for c in range(NCORES)],
                          axis=0)
    return outp.astype(np.float32), res.exec_time_ns


def kernel(**inputs) -> np.ndarray:
    out, _ = run(inputs, trace=False)
    return out


# revision 20
# speedup vs baseline: 1.0927x; 1.0927x over previous
"""GraphSAGE v4: dma_gather-based gathers + bf16 allgather.

Cross-core dedup of layer-1 rows (as v2): the 45,056 phase-2 refs are
deduped host-side to a sorted-unique list padded to 8 x SH rows; core c
computes h1 for its SH-row block.

Gathers use InstDMAGatherAnt (int16 indices, 16-partition-wrapped): the
SWDGE fixed cost (~1us) amortizes over thousands of descriptors, unlike
indirect_dma_start which on HW honors only one index per partition.
Phase-1 indices don't fit int16 against the 100K-row table, so the host
packs each 4-tile group's unique rows into a fixed 5632-row segment of a
per-core "bigtable" and rewrites indices group-locally (< 5632).

h1 is stored bf16: halves allgather bytes (CC fixed ~5us/chunk,
~158 GB/s) and phase-2 gather traffic. Aggregations run as one strided
tensor_reduce per tile on DVE; PSUM->SBUF copies go to the scalar
engine. Final rel err ~1e-3 (bf16 h1), well under the 2e-2 gate.
"""

import sys

for _p in ("/opt/trn_rl_repo", "/root/.axon_site/_ro/trn_rl_repo"):
    if _p not in sys.path:
        sys.path.insert(0, _p)

import numpy as np

import concourse.bass as bass
import concourse.mybir as mybir
import concourse.tile as tile
from concourse import bacc
from concourse.bass_utils import run_bass_kernel_spmd

N, D, OUT, K = 100000, 256, 128, 10
N1, B = 40960, 4096
NCORES = 8
BC = B // NCORES                 # 512 batch rows per core
NREF = BC * (K + 1)              # 5632 phase-2 refs
TR = NREF // 128                 # 44 phase-2 gather columns
T2 = BC // 128                   # 4 output tiles
K1 = K + 1
G = 4                            # phase-1 tiles per gather group
SEG = G * 128 * K1               # 5632: padded rows per bigtable segment
W1G = SEG // 16                  # 352: idx cols per full group (16-wrap)

_CACHE = {}


def _chunk_schedule(sh):
    """Allgather chunks (rows): ~1024-row chunks with a 512-row tail so
    the last chunk's serial latency (phase 2 waits on it) stays small."""
    chunks = []
    rem = sh
    while rem > 1536:
        chunks.append(1024)
        rem -= 1024
    if rem > 512:
        chunks.append(rem - 512)
        rem = 512
    chunks.append(rem)
    assert sum(chunks) == sh
    return tuple(chunks)


def _build(SH):
    T1 = SH // 128
    U = SH * NCORES
    NG = -(-T1 // G)
    CHUNKS = _chunk_schedule(SH)
    CH_START = tuple(sum(CHUNKS[:i]) for i in range(len(CHUNKS)))
    f32 = mybir.dt.float32
    bf16 = mybir.dt.bfloat16
    i16 = mybir.dt.int16
    nc = bacc.Bacc("TRN2", target_bir_lowering=False, debug=False,
                   num_devices=NCORES)
    bigtable = nc.dram_tensor("bigtable", [NG * SEG, D], f32,
                              kind="ExternalInput").ap()
    idx1 = nc.dram_tensor("idx1", [128, NG * W1G], i16,
                          kind="ExternalInput").ap()
    idx2 = nc.dram_tensor("idx2", [128, NREF // 16], i16,
                          kind="ExternalInput").ap()
    w1p = nc.dram_tensor("w1p", [2 * D, OUT], f32, kind="ExternalInput").ap()
    w2p = nc.dram_tensor("w2p", [2 * OUT, OUT], f32, kind="ExternalInput").ap()
    ident = nc.dram_tensor("ident", [128, 128], f32, kind="ExternalInput").ap()
    out = nc.dram_tensor("out", [BC, OUT], f32, kind="ExternalOutput").ap()
    shard = nc.dram_tensor("shard", [SH, OUT], bf16)
    h1all = nc.dram_tensor("h1all", [U, OUT], bf16, addr_space="Shared")

    relu = mybir.ActivationFunctionType.Relu

    with tile.TileContext(nc) as tc:
        with tc.tile_pool(name="const", bufs=1) as constp, \
             tc.tile_pool(name="gat", bufs=2) as gatp, \
             tc.tile_pool(name="agg", bufs=4) as aggp, \
             tc.tile_pool(name="xt", bufs=8) as xtp, \
             tc.tile_pool(name="g2", bufs=1) as g2p, \
             tc.tile_pool(name="ps", bufs=3, space="PSUM") as psp, \
             tc.tile_pool(name="psb", bufs=1, space="PSUM") as psbp, \
             tc.tile_pool(name="psh", bufs=2, space="PSUM") as pshp, \
             tc.tile_pool(name="o", bufs=4) as outp:

            # index tiles load first: HWDGE runs in program order per
            # engine, and the first gather only needs idx1's first group
            idx1_t = constp.tile([128, NG * W1G], i16, tag="idx1")
            nc.sync.dma_start(out=idx1_t[:, :W1G], in_=idx1[:, :W1G])
            if NG > 1:
                nc.sync.dma_start(out=idx1_t[:, W1G:], in_=idx1[:, W1G:])
            idn = constp.tile([128, 128], f32)
            nc.sync.dma_start(out=idn[:], in_=ident[:])
            idnb = constp.tile([128, 128], bf16, tag="idnb")
            nc.vector.tensor_copy(out=idnb[:], in_=idn[:])
            w1t = constp.tile([128, 4 * OUT], f32, tag="w1")
            for c in range(4):
                nc.sync.dma_start(out=w1t[:, c * OUT:(c + 1) * OUT],
                                  in_=w1p[c * 128:(c + 1) * 128, :])
            w2t = constp.tile([128, 2 * OUT], f32, tag="w2")
            for c in range(2):
                nc.sync.dma_start(out=w2t[:, c * OUT:(c + 1) * OUT],
                                  in_=w2p[c * 128:(c + 1) * 128, :])
            w2tb = constp.tile([128, 2 * OUT], bf16, tag="w2b")
            nc.vector.tensor_copy(out=w2tb[:], in_=w2t[:])
            idx2_t = constp.tile([128, NREF // 16], i16, tag="idx2")
            nc.sync.dma_start(out=idx2_t[:], in_=idx2[:, :])

            # ---- phase 1: compute node-major h1 shard -> DRAM ----
            for g in range(NG):
                gn = min(G, T1 - g * G)
                ncols = gn * K1
                g3 = gatp.tile([128, G * K1, D], f32)
                # HW caps one gather at 1024 indices (SWDGE desc scratch
                # split across the 8 Q7 cores) -> chunks of 8 columns
                for c0 in range(0, ncols, 8):
                    c1 = min(ncols, c0 + 8)
                    nc.gpsimd.dma_gather(
                        out_ap=g3[:, c0:c1, :],
                        in_ap=bigtable[g * SEG:(g + 1) * SEG, :],
                        idxs_ap=idx1_t[:, g * W1G + c0 * 8:g * W1G + c1 * 8],
                        num_idxs=(c1 - c0) * 128,
                        num_idxs_reg=(c1 - c0) * 128,
                        elem_size=D,
                    )
                for tl in range(gn):
                    t = g * G + tl
                    self2d = g3[:, tl * K1, :]
                    nbv = g3[:, tl * K1 + 1:(tl + 1) * K1, :].transpose(
                        [0, 2, 1])
                    a = aggp.tile([128, D], f32)
                    nc.vector.tensor_reduce(a[:], nbv, mybir.AxisListType.X,
                                            mybir.AluOpType.add)
                    srcs = (self2d[:, 0:128], self2d[:, 128:256],
                            a[:, 0:128], a[:, 128:256])
                    psum_h = pshp.tile([128, 128], f32, space="PSUM")
                    for c, src in enumerate(srcs):
                        pt = psp.tile([128, 128], f32, space="PSUM", tag="tp")
                        nc.tensor.transpose(out=pt[:], in_=src, identity=idn[:])
                        xt = xtp.tile([128, 128], f32, tag=f"xt{c}")
                        nc.scalar.copy(out=xt[:], in_=pt[:])
                        # node-major: out[nodes, outf] = xt.T @ w1chunk
                        nc.tensor.matmul(out=psum_h[:],
                                         lhsT=xt[:],
                                         rhs=w1t[:, c * OUT:(c + 1) * OUT],
                                         start=(c == 0), stop=(c == 3))
                    ho = outp.tile([128, OUT], bf16, tag="ho")
                    nc.scalar.activation(ho[:], psum_h[:], relu)
                    nc.sync.dma_start(out=shard[t * 128:(t + 1) * 128, :],
                                      in_=ho[:])

            # allgather after ALL phase-1 gathers: a collective blocks the
            # gpsimd queue on a cross-core barrier, and mid-stream that
            # stalls the Q7 descriptor pipeline (the critical resource at
            # ~8ns/descriptor). Issued here the barriers cost one skew.
            for (s, L) in zip(CH_START, CHUNKS):
                nc.gpsimd.collective_compute(
                    "AllGather", mybir.AluOpType.bypass,
                    replica_groups=[list(range(NCORES))],
                    ins=[shard[s:s + L, :]],
                    outs=[h1all[s * NCORES:(s + L) * NCORES, :]],
                )

            # ---- phase 2: gather h1 rows, aggregate, second layer ----
            # columns: [self(T2) | neighbors k-major (K*T2)]
            g2all = g2p.tile([128, TR, OUT], bf16, tag="g2all")
            for c0 in range(0, TR, 8):
                c1 = min(TR, c0 + 8)
                nc.gpsimd.dma_gather(
                    out_ap=g2all[:, c0:c1, :],
                    in_ap=h1all[:, :],
                    idxs_ap=idx2_t[:, c0 * 8:c1 * 8],
                    num_idxs=(c1 - c0) * 128,
                    num_idxs_reg=(c1 - c0) * 128,
                    elem_size=OUT,
                )
            g2v = g2all[:, :, :].rearrange("p (k2 t) o -> p k2 t o", k2=K1)
            for t in range(T2):
                a2 = aggp.tile([128, OUT], f32, tag="a2")
                nbv2 = g2v[:, 1:, t, :].transpose([0, 2, 1])
                nc.vector.tensor_reduce(a2[:], nbv2, mybir.AxisListType.X,
                                        mybir.AluOpType.add)
                self2 = g2v[:, 0, t, :]
                ps2 = pshp.tile([128, 128], f32, space="PSUM", tag="ps2")
                st = psbp.tile([128, 128], bf16, space="PSUM", tag="tpb")
                nc.tensor.transpose(out=st[:], in_=self2, identity=idnb[:])
                s2t = xtp.tile([128, 128], bf16, tag="s2t")
                nc.scalar.copy(out=s2t[:], in_=st[:])
                at = psp.tile([128, 128], f32, space="PSUM", tag="tp")
                nc.tensor.transpose(out=at[:], in_=a2[:], identity=idn[:])
                a2t = xtp.tile([128, 128], bf16, tag="a2t")
                nc.scalar.copy(out=a2t[:], in_=at[:])
                nc.tensor.matmul(out=ps2[:], lhsT=s2t[:], rhs=w2tb[:, 0:OUT],
                                 start=True, stop=False)
                nc.tensor.matmul(out=ps2[:], lhsT=a2t[:],
                                 rhs=w2tb[:, OUT:2 * OUT],
                                 start=False, stop=True)
                o = outp.tile([128, OUT], f32, tag="o2")
                nc.scalar.activation(o[:], ps2[:], relu)
                nc.sync.dma_start(out=out[t * 128:(t + 1) * 128, :], in_=o[:])

    nc.compile()
    return nc


def _wrap16(idxlist):
    """int16 index list -> [128, len/16] array with the 16-partition wrap
    the gather ucode expects: idx[p, s] = idxlist[s*16 + p], and the
    [16, W] pattern REPLICATED across all 8 Q7-core stripes (HW reads
    each core's own 16-partition stripe; zeros there silently gather
    row 0 instead)."""
    n = len(idxlist)
    assert n % 16 == 0
    w16 = idxlist.astype(np.int16).reshape(n // 16, 16).T
    return np.ascontiguousarray(np.tile(w16, (8, 1)))


def _prep_inputs(raw_features, W1, W2, nodes1, neighs1, map2, neighs2):
    raw = np.ascontiguousarray(np.asarray(raw_features, dtype=np.float32))
    W1 = np.asarray(W1, dtype=np.float32)
    W2 = np.asarray(W2, dtype=np.float32)
    nodes1 = np.asarray(nodes1).astype(np.int64)
    neighs1 = np.asarray(neighs1).astype(np.int64)
    map2 = np.asarray(map2).astype(np.int64)
    neighs2 = np.asarray(neighs2).astype(np.int64)

    w1p = np.concatenate([W1[:, :D], W1[:, D:] * (1.0 / K)], axis=1).T
    w2p = np.concatenate([W2[:, :OUT], W2[:, OUT:] * (1.0 / K)], axis=1).T
    w1p = np.ascontiguousarray(w1p, dtype=np.float32)
    w2p = np.ascontiguousarray(w2p, dtype=np.float32)
    ident = np.eye(128, dtype=np.float32)

    # global dedup of layer-1 rows over ALL cores; size the shard to fit
    refs = np.concatenate([map2, neighs2.reshape(-1)])      # [45056]
    uniq, inv = np.unique(refs, return_inverse=True)
    ua = len(uniq)
    SH = -(-ua // (NCORES * 128)) * 128  # per-core rows, 128-tile padded
    T1 = SH // 128
    U = SH * NCORES
    NG = -(-T1 // G)
    CHUNKS = _chunk_schedule(SH)
    CH_START = tuple(sum(CHUNKS[:i]) for i in range(len(CHUNKS)))
    uniq_pad = np.concatenate([uniq, np.zeros(U - ua, dtype=uniq.dtype)])
    # position of unique index u in the chunk-interleaved allgather layout
    cidx = np.arange(U) // SH            # owning core
    r = np.arange(U) % SH                # row within core shard
    starts = np.asarray(CH_START)
    sizes = np.asarray(CHUNKS)
    j = np.searchsorted(starts, r, side="right") - 1        # chunk id
    pos_of_u = starts[j] * NCORES + cidx * sizes[j] + (r - starts[j])

    in_maps = []
    for c in range(NCORES):
        # phase-1 raw row ids for this core's unique block: [SH, 11]
        blk = uniq_pad[c * SH:(c + 1) * SH]
        idsmat = np.stack([nodes1[blk]] + [neighs1[blk, k] for k in range(K)],
                          axis=1)                            # [SH, K1] int64
        # per-group compaction: unique rows -> fixed 5632-row segment,
        # indices rewritten group-local so they fit int16
        bigtable = np.zeros((NG * SEG, D), dtype=np.float32)
        idx1 = np.zeros((128, NG * W1G), dtype=np.int16)
        for g in range(NG):
            t0, t1 = g * G, min(T1, (g + 1) * G)
            gn = t1 - t0
            grefs = idsmat[t0 * 128:t1 * 128]                # [gn*128, K1]
            guniq, ginv = np.unique(grefs, return_inverse=True)
            assert len(guniq) <= SEG
            bigtable[g * SEG:g * SEG + len(guniq)] = raw[guniq]
            # gather order: (tile-local, k, partition)
            il = ginv.reshape(gn, 128, K1).transpose(0, 2, 1).reshape(-1)
            idx1[:, g * W1G:g * W1G + gn * K1 * 8] = _wrap16(il)
        # phase-2 refs: self (BC) then neighbors k-major (K x BC)
        sl = slice(c * BC, (c + 1) * BC)
        self_u = inv[np.arange(B)[sl]]                      # into uniq
        neigh_u = inv[B + (np.arange(c * BC * K, (c + 1) * BC * K)
                           .reshape(BC, K))]                # [BC, K]
        l2 = np.concatenate([pos_of_u[self_u],
                             pos_of_u[neigh_u.T.reshape(-1)]])
        idx2 = _wrap16(l2)
        in_maps.append({"bigtable": bigtable, "idx1": idx1, "idx2": idx2,
                        "w1p": w1p, "w2p": w2p, "ident": ident})
    return SH, in_maps


def run(inputs: dict, trace: bool = False):
    SH, in_maps = _prep_inputs(**inputs)
    if SH not in _CACHE:
        _CACHE[SH] = _build(SH)
    nc = _CACHE[SH]
    try:
        res = run_bass_kernel_spmd(nc, in_maps,
                                   core_ids=list(range(NCORES)), trace=trace)
    except Exception:
        # transient device wedge (e.g. NRT_EXEC_UNIT_UNRECOVERABLE) --
        # a single retry has been sufficient in practice
        res = run_bass_kernel_spmd(nc, in_maps,
                                   core_ids=list(range(NCORES)), trace=trace)
    outp = np.concatenate([res.results[c]["out"] for c in range(NCORES)],
                          axis=0)
    return outp.astype(np.float32), res.exec_time_ns


def kernel(**inputs) -> np.ndarray:
    out, _ = run(inputs, trace=False)
    return out


# revision 23
# speedup vs baseline: 1.7053x; 1.5607x over previous
"""GraphSAGE v4: dma_gather-based gathers + bf16 allgather.

Cross-core dedup of layer-1 rows (as v2): the 45,056 phase-2 refs are
deduped host-side to a sorted-unique list padded to 8 x SH rows; core c
computes h1 for its SH-row block.

Gathers use InstDMAGatherAnt (int16 indices, 16-partition-wrapped): the
SWDGE fixed cost (~1us) amortizes over thousands of descriptors, unlike
indirect_dma_start which on HW honors only one index per partition.
Phase-1 indices don't fit int16 against the 100K-row table, so the host
packs each 4-tile group's unique rows into a fixed 5632-row segment of a
per-core "bigtable" and rewrites indices group-locally (< 5632).

h1 is stored bf16: halves allgather bytes (CC fixed ~5us/chunk,
~158 GB/s) and phase-2 gather traffic. Aggregations run as one strided
tensor_reduce per tile on DVE; PSUM->SBUF copies go to the scalar
engine. Final rel err ~1e-3 (bf16 h1), well under the 2e-2 gate.
"""

import sys

for _p in ("/opt/trn_rl_repo", "/root/.axon_site/_ro/trn_rl_repo"):
    if _p not in sys.path:
        sys.path.insert(0, _p)

import numpy as np

import concourse.bass as bass
import concourse.mybir as mybir
import concourse.tile as tile
from concourse import bacc
from concourse.bass_utils import run_bass_kernel_spmd

N, D, OUT, K = 100000, 256, 128, 10
N1, B = 40960, 4096
NCORES = 8
BC = B // NCORES                 # 512 batch rows per core
NREF = BC * (K + 1)              # 5632 phase-2 refs
TR = NREF // 128                 # 44 phase-2 gather columns
T2 = BC // 128                   # 4 output tiles
K1 = K + 1
G = 4                            # phase-1 tiles per gather group
SEG = G * 128 * K1               # 5632: padded rows per bigtable segment
W1G = SEG // 16                  # 352: idx cols per full group (16-wrap)

_CACHE = {}


def _chunk_schedule(sh):
    """Allgather chunks (rows): ~1024-row chunks with a 512-row tail so
    the last chunk's serial latency (phase 2 waits on it) stays small."""
    chunks = []
    rem = sh
    while rem > 1536:
        chunks.append(1024)
        rem -= 1024
    if rem > 512:
        chunks.append(rem - 512)
        rem = 512
    chunks.append(rem)
    assert sum(chunks) == sh
    return tuple(chunks)


def _build(SH):
    T1 = SH // 128
    U = SH * NCORES
    NG = -(-T1 // G)
    CHUNKS = _chunk_schedule(SH)
    CH_START = tuple(sum(CHUNKS[:i]) for i in range(len(CHUNKS)))
    f32 = mybir.dt.float32
    bf16 = mybir.dt.bfloat16
    i16 = mybir.dt.int16
    nc = bacc.Bacc("TRN2", target_bir_lowering=False, debug=False,
                   num_devices=NCORES, num_swdge_queues=4)
    bigtable = nc.dram_tensor("bigtable", [NG * SEG, D], f32,
                              kind="ExternalInput").ap()
    idx1 = nc.dram_tensor("idx1", [128, NG * W1G], i16,
                          kind="ExternalInput").ap()
    idx2 = nc.dram_tensor("idx2", [128, NREF // 16], i16,
                          kind="ExternalInput").ap()
    w1p = nc.dram_tensor("w1p", [2 * D, OUT], f32, kind="ExternalInput").ap()
    w2p = nc.dram_tensor("w2p", [2 * OUT, OUT], f32, kind="ExternalInput").ap()
    ident = nc.dram_tensor("ident", [128, 128], f32, kind="ExternalInput").ap()
    out = nc.dram_tensor("out", [BC, OUT], f32, kind="ExternalOutput").ap()
    shard = nc.dram_tensor("shard", [SH, OUT], bf16)
    h1all = nc.dram_tensor("h1all", [U, OUT], bf16, addr_space="Shared")

    relu = mybir.ActivationFunctionType.Relu

    with tile.TileContext(nc) as tc:
        with tc.tile_pool(name="const", bufs=1) as constp, \
             tc.tile_pool(name="gat", bufs=2) as gatp, \
             tc.tile_pool(name="agg", bufs=4) as aggp, \
             tc.tile_pool(name="xt", bufs=8) as xtp, \
             tc.tile_pool(name="g2", bufs=1) as g2p, \
             tc.tile_pool(name="ps", bufs=3, space="PSUM") as psp, \
             tc.tile_pool(name="psb", bufs=1, space="PSUM") as psbp, \
             tc.tile_pool(name="psh", bufs=2, space="PSUM") as pshp, \
             tc.tile_pool(name="o", bufs=4) as outp:

            # index tiles load first: HWDGE runs in program order per
            # engine, and the first gather only needs idx1's first group
            idx1_t = constp.tile([128, NG * W1G], i16, tag="idx1")
            nc.sync.dma_start(out=idx1_t[:, :W1G], in_=idx1[:, :W1G])
            if NG > 1:
                nc.sync.dma_start(out=idx1_t[:, W1G:], in_=idx1[:, W1G:])
            idn = constp.tile([128, 128], f32)
            nc.sync.dma_start(out=idn[:], in_=ident[:])
            idnb = constp.tile([128, 128], bf16, tag="idnb")
            nc.vector.tensor_copy(out=idnb[:], in_=idn[:])
            w1t = constp.tile([128, 4 * OUT], f32, tag="w1")
            for c in range(4):
                nc.sync.dma_start(out=w1t[:, c * OUT:(c + 1) * OUT],
                                  in_=w1p[c * 128:(c + 1) * 128, :])
            w2t = constp.tile([128, 2 * OUT], f32, tag="w2")
            for c in range(2):
                nc.sync.dma_start(out=w2t[:, c * OUT:(c + 1) * OUT],
                                  in_=w2p[c * 128:(c + 1) * 128, :])
            w2tb = constp.tile([128, 2 * OUT], bf16, tag="w2b")
            nc.vector.tensor_copy(out=w2tb[:], in_=w2t[:])
            idx2_t = constp.tile([128, NREF // 16], i16, tag="idx2")
            nc.sync.dma_start(out=idx2_t[:], in_=idx2[:, :])

            # ---- phase 1: compute node-major h1 shard -> DRAM ----
            qn = 0
            for g in range(NG):
                gn = min(G, T1 - g * G)
                ncols = gn * K1
                g3 = gatp.tile([128, G * K1, D], f32)
                # HW caps one gather at 1024 indices (SWDGE desc scratch
                # split across the 8 Q7 cores) -> chunks of 8 columns;
                # rotate the 4 SWDGE queues to overlap per-instr overhead
                for c0 in range(0, ncols, 8):
                    c1 = min(ncols, c0 + 8)
                    nc.gpsimd.dma_gather(
                        out_ap=g3[:, c0:c1, :],
                        in_ap=bigtable[g * SEG:(g + 1) * SEG, :],
                        idxs_ap=idx1_t[:, g * W1G + c0 * 8:g * W1G + c1 * 8],
                        num_idxs=(c1 - c0) * 128,
                        num_idxs_reg=(c1 - c0) * 128,
                        elem_size=D,
                        queue_num=qn,
                    )
                    qn = (qn + 1) % 4
                for tl in range(gn):
                    t = g * G + tl
                    self2d = g3[:, tl * K1, :]
                    nbv = g3[:, tl * K1 + 1:(tl + 1) * K1, :].transpose(
                        [0, 2, 1])
                    a = aggp.tile([128, D], f32)
                    nc.vector.tensor_reduce(a[:], nbv, mybir.AxisListType.X,
                                            mybir.AluOpType.add)
                    srcs = (self2d[:, 0:128], self2d[:, 128:256],
                            a[:, 0:128], a[:, 128:256])
                    psum_h = pshp.tile([128, 128], f32, space="PSUM")
                    for c, src in enumerate(srcs):
                        pt = psp.tile([128, 128], f32, space="PSUM", tag="tp")
                        nc.tensor.transpose(out=pt[:], in_=src, identity=idn[:])
                        xt = xtp.tile([128, 128], f32, tag=f"xt{c}")
                        nc.scalar.copy(out=xt[:], in_=pt[:])
                        # node-major: out[nodes, outf] = xt.T @ w1chunk
                        nc.tensor.matmul(out=psum_h[:],
                                         lhsT=xt[:],
                                         rhs=w1t[:, c * OUT:(c + 1) * OUT],
                                         start=(c == 0), stop=(c == 3))
                    ho = outp.tile([128, OUT], bf16, tag="ho")
                    nc.scalar.activation(ho[:], psum_h[:], relu)
                    nc.sync.dma_start(out=shard[t * 128:(t + 1) * 128, :],
                                      in_=ho[:])

            # allgather after ALL phase-1 gathers: a collective blocks the
            # gpsimd queue on a cross-core barrier, and mid-stream that
            # stalls the Q7 descriptor pipeline (the critical resource at
            # ~8ns/descriptor). Issued here the barriers cost one skew.
            for (s, L) in zip(CH_START, CHUNKS):
                nc.gpsimd.collective_compute(
                    "AllGather", mybir.AluOpType.bypass,
                    replica_groups=[list(range(NCORES))],
                    ins=[shard[s:s + L, :]],
                    outs=[h1all[s * NCORES:(s + L) * NCORES, :]],
                )

            # ---- phase 2: gather h1 rows, aggregate, second layer ----
            # columns: [self(T2) | neighbors k-major (K*T2)]
            g2all = g2p.tile([128, TR, OUT], bf16, tag="g2all")
            for c0 in range(0, TR, 8):
                c1 = min(TR, c0 + 8)
                nc.gpsimd.dma_gather(
                    out_ap=g2all[:, c0:c1, :],
                    in_ap=h1all[:, :],
                    idxs_ap=idx2_t[:, c0 * 8:c1 * 8],
                    num_idxs=(c1 - c0) * 128,
                    num_idxs_reg=(c1 - c0) * 128,
                    elem_size=OUT,
                    queue_num=qn,
                )
                qn = (qn + 1) % 4
            g2v = g2all[:, :, :].rearrange("p (k2 t) o -> p k2 t o", k2=K1)
            for t in range(T2):
                a2 = aggp.tile([128, OUT], f32, tag="a2")
                nbv2 = g2v[:, 1:, t, :].transpose([0, 2, 1])
                nc.vector.tensor_reduce(a2[:], nbv2, mybir.AxisListType.X,
                                        mybir.AluOpType.add)
                self2 = g2v[:, 0, t, :]
                ps2 = pshp.tile([128, 128], f32, space="PSUM", tag="ps2")
                st = psbp.tile([128, 128], bf16, space="PSUM", tag="tpb")
                nc.tensor.transpose(out=st[:], in_=self2, identity=idnb[:])
                s2t = xtp.tile([128, 128], bf16, tag="s2t")
                nc.scalar.copy(out=s2t[:], in_=st[:])
                at = psp.tile([128, 128], f32, space="PSUM", tag="tp")
                nc.tensor.transpose(out=at[:], in_=a2[:], identity=idn[:])
                a2t = xtp.tile([128, 128], bf16, tag="a2t")
                nc.scalar.copy(out=a2t[:], in_=at[:])
                nc.tensor.matmul(out=ps2[:], lhsT=s2t[:], rhs=w2tb[:, 0:OUT],
                                 start=True, stop=False)
                nc.tensor.matmul(out=ps2[:], lhsT=a2t[:],
                                 rhs=w2tb[:, OUT:2 * OUT],
                                 start=False, stop=True)
                o = outp.tile([128, OUT], f32, tag="o2")
                nc.scalar.activation(o[:], ps2[:], relu)
                nc.sync.dma_start(out=out[t * 128:(t + 1) * 128, :], in_=o[:])

    nc.compile()
    return nc


def _wrap16(idxlist):
    """int16 index list -> [128, len/16] array with the 16-partition wrap
    the gather ucode expects: idx[p, s] = idxlist[s*16 + p], and the
    [16, W] pattern REPLICATED across all 8 Q7-core stripes (HW reads
    each core's own 16-partition stripe; zeros there silently gather
    row 0 instead)."""
    n = len(idxlist)
    assert n % 16 == 0
    w16 = idxlist.astype(np.int16).reshape(n // 16, 16).T
    return np.ascontiguousarray(np.tile(w16, (8, 1)))


def _prep_inputs(raw_features, W1, W2, nodes1, neighs1, map2, neighs2):
    raw = np.ascontiguousarray(np.asarray(raw_features, dtype=np.float32))
    W1 = np.asarray(W1, dtype=np.float32)
    W2 = np.asarray(W2, dtype=np.float32)
    nodes1 = np.asarray(nodes1).astype(np.int64)
    neighs1 = np.asarray(neighs1).astype(np.int64)
    map2 = np.asarray(map2).astype(np.int64)
    neighs2 = np.asarray(neighs2).astype(np.int64)

    w1p = np.concatenate([W1[:, :D], W1[:, D:] * (1.0 / K)], axis=1).T
    w2p = np.concatenate([W2[:, :OUT], W2[:, OUT:] * (1.0 / K)], axis=1).T
    w1p = np.ascontiguousarray(w1p, dtype=np.float32)
    w2p = np.ascontiguousarray(w2p, dtype=np.float32)
    ident = np.eye(128, dtype=np.float32)

    # global dedup of layer-1 rows over ALL cores; size the shard to fit
    refs = np.concatenate([map2, neighs2.reshape(-1)])      # [45056]
    uniq, inv = np.unique(refs, return_inverse=True)
    ua = len(uniq)
    SH = -(-ua // (NCORES * 128)) * 128  # per-core rows, 128-tile padded
    T1 = SH // 128
    U = SH * NCORES
    NG = -(-T1 // G)
    CHUNKS = _chunk_schedule(SH)
    CH_START = tuple(sum(CHUNKS[:i]) for i in range(len(CHUNKS)))
    uniq_pad = np.concatenate([uniq, np.zeros(U - ua, dtype=uniq.dtype)])
    # position of unique index u in the chunk-interleaved allgather layout
    cidx = np.arange(U) // SH            # owning core
    r = np.arange(U) % SH                # row within core shard
    starts = np.asarray(CH_START)
    sizes = np.asarray(CHUNKS)
    j = np.searchsorted(starts, r, side="right") - 1        # chunk id
    pos_of_u = starts[j] * NCORES + cidx * sizes[j] + (r - starts[j])

    in_maps = []
    for c in range(NCORES):
        # phase-1 raw row ids for this core's unique block: [SH, 11]
        blk = uniq_pad[c * SH:(c + 1) * SH]
        idsmat = np.stack([nodes1[blk]] + [neighs1[blk, k] for k in range(K)],
                          axis=1)                            # [SH, K1] int64
        # per-group compaction: unique rows -> fixed 5632-row segment,
        # indices rewritten group-local so they fit int16
        bigtable = np.zeros((NG * SEG, D), dtype=np.float32)
        idx1 = np.zeros((128, NG * W1G), dtype=np.int16)
        for g in range(NG):
            t0, t1 = g * G, min(T1, (g + 1) * G)
            gn = t1 - t0
            grefs = idsmat[t0 * 128:t1 * 128]                # [gn*128, K1]
            guniq, ginv = np.unique(grefs, return_inverse=True)
            assert len(guniq) <= SEG
            bigtable[g * SEG:g * SEG + len(guniq)] = raw[guniq]
            # gather order: (tile-local, k, partition)
            il = ginv.reshape(gn, 128, K1).transpose(0, 2, 1).reshape(-1)
            idx1[:, g * W1G:g * W1G + gn * K1 * 8] = _wrap16(il)
        # phase-2 refs: self (BC) then neighbors k-major (K x BC)
        sl = slice(c * BC, (c + 1) * BC)
        self_u = inv[np.arange(B)[sl]]                      # into uniq
        neigh_u = inv[B + (np.arange(c * BC * K, (c + 1) * BC * K)
                           .reshape(BC, K))]                # [BC, K]
        l2 = np.concatenate([pos_of_u[self_u],
                             pos_of_u[neigh_u.T.reshape(-1)]])
        idx2 = _wrap16(l2)
        in_maps.append({"bigtable": bigtable, "idx1": idx1, "idx2": idx2,
                        "w1p": w1p, "w2p": w2p, "ident": ident})
    return SH, in_maps


def run(inputs: dict, trace: bool = False):
    SH, in_maps = _prep_inputs(**inputs)
    if SH not in _CACHE:
        _CACHE[SH] = _build(SH)
    nc = _CACHE[SH]
    try:
        res = run_bass_kernel_spmd(nc, in_maps,
                                   core_ids=list(range(NCORES)), trace=trace)
    except Exception:
        # transient device wedge (e.g. NRT_EXEC_UNIT_UNRECOVERABLE) --
        # a single retry has been sufficient in practice
        res = run_bass_kernel_spmd(nc, in_maps,
                                   core_ids=list(range(NCORES)), trace=trace)
    outp = np.concatenate([res.results[c]["out"] for c in range(NCORES)],
                          axis=0)
    return outp.astype(np.float32), res.exec_time_ns


def kernel(**inputs) -> np.ndarray:
    out, _ = run(inputs, trace=False)
    return out
